# revision 19
# baseline (speedup 1.0000x reference)
"""Bahdanau-style attention kernel for TRN2, SPMD over 8 NeuronCores.

Reference computation (per example n):
    ke    = h_enc[n] @ Wk.T + bk                  [L, E]
    qe    = h_dec[n] @ Wq.T + bq                  [E]
    attn  = tanh(qe @ Wh_q.T + ke @ Wh_k.T + bh)  [L, A]
    score = attn @ v ; masked softmax -> weights  [L]
    ctx   = weights @ ke                          [E]

Algebraic reformulation used here (all exact up to f32 rounding):
    S     = h_enc[n] @ Wc + beta[n],  Wc = Wk.T @ Wh_k.T (host, f64)
            beta[n] = Wh_q @ qe[n] + Wh_k @ bk + bh      (host, f64)
    score = tanh(S) @ v ; softmax -> weights
    u     = weights @ h_enc[n]                    [H]
    ctx   = u @ Wk.T + bk                          (bk added on host)

so the big [L,E] intermediate `ke` is never materialized.

Sharding: batch N=64 -> 8 cores x 8 "slots". Examples are dealt to slots
sorted by tile count (nt = ceil(src_len/128)) so each slot's static tile
count (max over its 8 cores) is close to each member's own count. Masked
positions get score -1e10 via a data-side mask, making their softmax
weights exactly 0.0 (matching the reference bit-for-bit in f32), so extra
tiles computed by cores with shorter examples are harmless.
"""

import numpy as np

N_FULL, L_FULL, H, E, A = 64, 2048, 1024, 512, 512
P = 128           # partition dim / l-tile size
NCORES = 8
SLOTS = 8         # examples per core
NEG = -1e10

_prog_cache = {}


def _build_program(ntk, slots=SLOTS, debug=False, prec="fp16"):
    """Build one SPMD bass program for slot tile counts ntk (tuple of len slots)."""
    import concourse.bass as bass
    import concourse.bacc as bacc
    import concourse.mybir as mybir
    from concourse import tile, masks

    dt = mybir.dt.float32
    dtb = {
        "fp16": mybir.dt.float16,
        "bf16": mybir.dt.bfloat16,
        "fp32": mybir.dt.float32,
    }[prec]
    AF = mybir.ActivationFunctionType
    OP = mybir.AluOpType
    AX = mybir.AxisListType

    nc = bacc.Bacc(
        "TRN2",
        target_bir_lowering=False,
        debug=debug,
        num_devices=NCORES,
    )

    henc_in = nc.declare_dram_parameter("henc", [slots, L_FULL, H], dtb, isOutput=False)
    # pre-transposed h_enc, host layout [slot, l-tile, p=h%128, (j=h//128, l%128)]
    hencT_in = nc.declare_dram_parameter(
        "hencT", [slots, L_FULL // P, P, H], dtb, isOutput=False
    )
    mask_in = nc.declare_dram_parameter("maskT", [slots, P, L_FULL // P], dt, isOutput=False)
    beta_in = nc.declare_dram_parameter("beta", [slots, P, A], dt, isOutput=False)
    wc_in = nc.declare_dram_parameter("wc", [P, (H // P) * A], dtb, isOutput=False)
    wkT_in = nc.declare_dram_parameter("wkT", [P, (H // P) * E], dt, isOutput=False)
    vb_in = nc.declare_dram_parameter("vb", [P, A], dt, isOutput=False)
    ones_in = nc.declare_dram_parameter("onesr", [1, P], dt, isOutput=False)
    wout = nc.declare_dram_parameter("wout", [slots, L_FULL // P, P], dt, isOutput=True)
    ctx_out = nc.declare_dram_parameter("ctxout", [slots, E], dt, isOutput=True)

    with tile.TileContext(nc) as tc:
        with (
            tc.tile_pool(name="const", bufs=1) as constp,
            tc.tile_pool(name="x", bufs=20) as xpool,
            tc.tile_pool(name="xT", bufs=6) as xTpool,
            tc.tile_pool(name="th", bufs=3) as thpool,
            tc.tile_pool(name="junk", bufs=2) as junkpool,
            tc.tile_pool(name="scol", bufs=2) as scolp,
            tc.tile_pool(name="small", bufs=8) as smallp,
            tc.tile_pool(name="psS", bufs=3, space="PSUM") as psS,
            tc.tile_pool(name="psU", bufs=1, space="PSUM") as psU,
            tc.tile_pool(name="psE", bufs=2, space="PSUM") as psE,
        ):
            # ---- persistent constants ----
            wc_sb = constp.tile([P, (H // P) * A], dtb)
            nc.sync.dma_start(wc_sb[:], wc_in[:])
            wkT_sb = constp.tile([P, (H // P) * E], dt)
            nc.sync.dma_start(wkT_sb[:], wkT_in[:])
            v_sb = constp.tile([P, A], dt)
            nc.sync.dma_start(v_sb[:], vb_in[:])
            ones_sb = constp.tile([1, P], dt)
            nc.sync.dma_start(ones_sb[:], ones_in[:])
            ident = constp.tile([P, P], dt)
            masks.make_identity(nc, ident[:])
            ones_col = constp.tile([P, 1], dt)
            nc.vector.memset(ones_col[:], 1.0)
            u_all = constp.tile([slots, H], dt)

            for k in range(slots):
                nt = ntk[k]
                beta_sb = smallp.tile([P, A], dt, tag="beta")
                nc.sync.dma_start(beta_sb[:], beta_in[k])
                maskT_sb = smallp.tile([P, L_FULL // P], dt, tag="mask")
                nc.sync.dma_start(maskT_sb[:], mask_in[k])

                scol = scolp.tile([P, nt], dt, tag="scol")
                xts = []
                for t in range(nt):
                    x = xpool.tile([P, H], dtb)
                    xts.append(x)
                    nc.gpsimd.dma_start(x[:], henc_in[k, t * P : (t + 1) * P, :])
                    xT = xTpool.tile([P, H], dtb, tag="xT")
                    nc.sync.dma_start(xT[:], hencT_in[k, t])
                    # score matmul: S = beta (rank-1) + sum_j xT_j.T @ Wc_j
                    s_ps = psS.tile([P, A], dt, tag="s")
                    for j in range(H // P):
                        nc.tensor.matmul(
                            s_ps[:],
                            xT[:, j * P : (j + 1) * P],
                            wc_sb[:, j * A : (j + 1) * A],
                            start=(j == 0),
                            stop=(j == H // P - 1),
                        )
                    nc.vector.scalar_tensor_tensor(
                        out=s_ps[:], in0=s_ps[:], scalar=0.0, in1=beta_sb[:],
                        op0=OP.add, op1=OP.add,
                    )
                    th = thpool.tile([P, A], dt, tag="th")
                    nc.scalar.activation(th[:], s_ps[:], AF.Tanh)
                    junk = junkpool.tile([P, A], dt, tag="junk")
                    nc.vector.scalar_tensor_tensor(
                        out=junk[:],
                        in0=th[:],
                        scalar=0.0,
                        in1=v_sb[:],
                        op0=OP.add,
                        op1=OP.mult,
                        accum_out=scol[:, t : t + 1],
                    )

                # ---- slot epilogue: masked softmax over [P, nt] ----
                nc.vector.scalar_tensor_tensor(
                    out=scol[:], in0=scol[:], scalar=0.0, in1=maskT_sb[:, :nt],
                    op0=OP.add, op1=OP.add,
                )
                colmax = smallp.tile([P, 1], dt, tag="colmax")
                nc.vector.tensor_reduce(colmax[:], scol[:], axis=AX.X, op=OP.max)
                tmax_ps = psE.tile([1, P], dt, tag="e")
                nc.tensor.transpose(tmax_ps[:], colmax[:], ident[:])
                negm = smallp.tile([1, 1], dt, tag="negm")
                nc.vector.tensor_reduce(negm[:], tmax_ps[:], axis=AX.X, op=OP.max, negate=True)
                negm_ps = psE.tile([P, 1], dt, tag="e")
                nc.tensor.matmul(negm_ps[:], ones_sb[:], negm[:], start=True, stop=True)
                negm128 = smallp.tile([P, 1], dt, tag="negm128")
                nc.scalar.copy(negm128[:], negm_ps[:])

                pmat = scolp.tile([P, nt], dt, tag="pmat")
                rowsum = smallp.tile([P, 1], dt, tag="rowsum")
                nc.scalar.activation(
                    pmat[:], scol[:], AF.Exp, bias=negm128[:], scale=1.0,
                    accum_out=rowsum[:],
                )
                z_ps = psE.tile([1, 1], dt, tag="e")
                nc.tensor.matmul(z_ps[:], rowsum[:], ones_col[:], start=True, stop=True)
                z_sb = smallp.tile([1, 1], dt, tag="z_sb")
                nc.scalar.copy(z_sb[:], z_ps[:])
                rz = smallp.tile([1, 1], dt, tag="rz")
                nc.vector.reciprocal(rz[:], z_sb[:])
                rz_ps = psE.tile([P, 1], dt, tag="e")
                nc.tensor.matmul(rz_ps[:], ones_sb[:], rz[:], start=True, stop=True)
                rz128 = smallp.tile([P, 1], dt, tag="rz128")
                nc.scalar.copy(rz128[:], rz_ps[:])

                wn = scolp.tile([P, nt], dt, tag="wn")
                nc.vector.tensor_scalar_mul(wn[:], pmat[:], rz128[:])
                wT_ps = psE.tile([nt, P], dt, tag="e")
                nc.tensor.matmul(wT_ps[:], wn[:], ident[:], start=True, stop=True, is_transpose=True)
                wT_sb = smallp.tile([nt, P], dt, tag="wT_sb")
                nc.scalar.copy(wT_sb[:], wT_ps[:])
                nc.sync.dma_start(wout[k, :nt, :], wT_sb[:])

                # ---- u accumulation: u = sum_t P[:,t].T @ x_t  (unnormalized) ----
                pmat16 = scolp.tile([P, nt], dtb, tag="pmat16")
                nc.vector.tensor_copy(pmat16[:], pmat[:])
                u_ps = [
                    psU.tile([1, E], dt, tag=f"u{half}", name=f"u_ps{half}_{k}")
                    for half in range(2)
                ]
                for t in range(nt):
                    for half in range(2):
                        nc.tensor.matmul(
                            u_ps[half][:],
                            pmat16[:, t : t + 1],
                            xts[t][:, half * E : (half + 1) * E],
                            start=(t == 0),
                            stop=(t == nt - 1),
                        )
                u_row = smallp.tile([1, H], dt, tag="u_row")
                for half in range(2):
                    nc.scalar.activation(
                        u_row[:, half * E : (half + 1) * E],
                        u_ps[half][:],
                        AF.Copy,
                        scale=rz[:],
                    )
                nc.sync.dma_start(u_all[k : k + 1, :], u_row[:])

            # ---- ctx epilogue: ctx = (U @ WkT) scaled rows already ----
            ctx_ps = psE.tile([slots, E], dt, tag="e")
            for j in range(H // P):
                ut_ps = psE.tile([P, slots], dt, tag="e")
                nc.tensor.matmul(
                    ut_ps[:], u_all[:, j * P : (j + 1) * P], ident[:slots, :slots],
                    start=True, stop=True, is_transpose=True,
                )
                utj = smallp.tile([P, slots], dt, tag="utj")
                nc.vector.tensor_copy(utj[:], ut_ps[:])
                nc.tensor.matmul(
                    ctx_ps[:], utj[:], wkT_sb[:, j * E : (j + 1) * E],
                    start=(j == 0), stop=(j == H // P - 1),
                )
            ctx_sb = constp.tile([slots, E], dt)
            nc.scalar.copy(ctx_sb[:], ctx_ps[:])
            nc.sync.dma_start(ctx_out[:], ctx_sb[:])

    nc.compile()
    return nc


def kernel(h_dec, h_enc, src_lens, Wq, bq, Wk, bk, Wh, bh, v):
    import os
    import ml_dtypes
    from concourse.bass_utils import run_bass_kernel_spmd

    h_dec = np.asarray(h_dec, dtype=np.float32)
    h_enc = np.ascontiguousarray(np.asarray(h_enc, dtype=np.float32))
    src_lens_np = np.asarray(src_lens)
    Wq = np.asarray(Wq, dtype=np.float32); bq = np.asarray(bq, dtype=np.float32)
    Wk = np.asarray(Wk, dtype=np.float32); bk = np.asarray(bk, dtype=np.float32)
    Wh = np.asarray(Wh, dtype=np.float32); bh = np.asarray(bh, dtype=np.float32)
    v = np.asarray(v, dtype=np.float32)

    Wh_q, Wh_k = Wh[:, :E], Wh[:, E:]
    Wc = np.ascontiguousarray(
        (Wk.T.astype(np.float64) @ Wh_k.T.astype(np.float64)).astype(np.float32)
    )  # [H, A]
    qe = h_dec.astype(np.float64) @ Wq.T.astype(np.float64) + bq.astype(np.float64)
    beta = (
        qe @ Wh_q.T.astype(np.float64)
        + Wh_k.astype(np.float64) @ bk.astype(np.float64)
        + bh.astype(np.float64)
    ).astype(np.float32)  # [N, A]
    WkT = np.ascontiguousarray(Wk.T)  # [H, E]

    lens = np.clip(src_lens_np.astype(np.int64), 1, L_FULL)
    nt = np.ceil(lens / P).astype(np.int64)  # [N] tiles per example

    # deal examples (sorted by nt desc) into slots: slot k <- ranks [8k, 8k+8)
    order = np.argsort(-nt, kind="stable")
    assign = order.reshape(SLOTS, NCORES)          # assign[k, c] = example id
    ntk = tuple(int(nt[assign[k]].max()) for k in range(SLOTS))

    prec = os.environ.get("KERNEL_PREC", "fp16")
    key = (ntk, prec)
    if key not in _prog_cache:
        _prog_cache[key] = _build_program(ntk, prec=prec)
    nc = _prog_cache[key]

    # mask[n]: [P, L/P] with mask[p, t] = 0 if (t*P + p) < len else NEG
    l_idx = np.arange(L_FULL).reshape(L_FULL // P, P).T  # [P, L/P]
    mmdt = {"fp16": np.float16, "bf16": ml_dtypes.bfloat16, "fp32": np.float32}[prec]
    wc_dev = np.ascontiguousarray(
        Wc.reshape(H // P, P, A).transpose(1, 0, 2).reshape(P, (H // P) * A)
    ).astype(mmdt)
    wkT_dev = np.ascontiguousarray(
        WkT.reshape(H // P, P, E).transpose(1, 0, 2).reshape(P, (H // P) * E)
    )
    v_dev = np.ascontiguousarray(np.broadcast_to(v, (P, A)))
    ones_dev = np.ones((1, P), dtype=np.float32)

    in_maps = []
    for c in range(NCORES):
        ex = assign[:, c]  # [SLOTS]
        maskT = np.where(l_idx[None, :, :] < lens[ex][:, None, None], 0.0, NEG).astype(np.float32)
        in_maps.append({
            "henc": np.ascontiguousarray(h_enc[ex]).astype(mmdt),
            "hencT": np.ascontiguousarray(
                h_enc[ex]
                .reshape(SLOTS, L_FULL // P, P, H // P, P)
                .transpose(0, 1, 4, 3, 2)
                .reshape(SLOTS, L_FULL // P, P, H)
            ).astype(mmdt),
            "maskT": np.ascontiguousarray(maskT),
            "beta": np.ascontiguousarray(
                np.broadcast_to(beta[ex][:, None, :], (SLOTS, P, A))
            ),
            "wc": wc_dev,
            "wkT": wkT_dev,
            "vb": v_dev,
            "onesr": ones_dev,
        })

    trace_dir = os.environ.get("KERNEL_TRACE_DIR")
    if trace_dir:
        import axon_profile_shim
        axon_profile_shim.install()
        kr = run_bass_kernel_spmd(
            nc, in_maps, list(range(NCORES)), trace=True, tmpdir=trace_dir
        )
        global LAST_RESULTS
        LAST_RESULTS = kr
        print(f"HW exec time: {kr.exec_time_ns} ns (mean {kr.mean_exec_time_ns})")
        res = kr.results
    else:
        res = run_bass_kernel_spmd(nc, in_maps, list(range(NCORES))).results

    ctx = np.zeros((N_FULL, E), dtype=np.float32)
    weights = np.zeros((N_FULL, L_FULL), dtype=np.float32)
    for c in range(NCORES):
        w_c = res[c]["wout"].reshape(SLOTS, L_FULL)
        ctx_c = res[c]["ctxout"]
        for k in range(SLOTS):
            n = assign[k, c]
            m = ntk[k] * P
            weights[n, :m] = w_c[k, :m]
            ctx[n] = ctx_c[k] + bk
    return ctx, weights


# revision 20
# speedup vs baseline: 1.0527x; 1.0527x over previous
"""Bahdanau-style attention kernel for TRN2, SPMD over 8 NeuronCores.

Reference computation (per example n):
    ke    = h_enc[n] @ Wk.T + bk                  [L, E]
    qe    = h_dec[n] @ Wq.T + bq                  [E]
    attn  = tanh(qe @ Wh_q.T + ke @ Wh_k.T + bh)  [L, A]
    score = attn @ v ; masked softmax -> weights  [L]
    ctx   = weights @ ke                          [E]

Algebraic reformulation used here (all exact up to f32 rounding):
    S     = h_enc[n] @ Wc + beta[n],  Wc = Wk.T @ Wh_k.T (host, f64)
            beta[n] = Wh_q @ qe[n] + Wh_k @ bk + bh      (host, f64)
    score = tanh(S) @ v ; softmax -> weights
    u     = weights @ h_enc[n]                    [H]
    ctx   = u @ Wk.T + bk                          (bk added on host)

so the big [L,E] intermediate `ke` is never materialized.

Sharding: batch N=64 -> 8 cores x 8 "slots". Examples are dealt to slots
sorted by tile count (nt = ceil(src_len/128)) so each slot's static tile
count (max over its 8 cores) is close to each member's own count. Masked
positions get score -1e10 via a data-side mask, making their softmax
weights exactly 0.0 (matching the reference bit-for-bit in f32), so extra
tiles computed by cores with shorter examples are harmless.
"""

import numpy as np

N_FULL, L_FULL, H, E, A = 64, 2048, 1024, 512, 512
P = 128           # partition dim / l-tile size
NCORES = 8
SLOTS = 8         # examples per core
NEG = -1e10

_prog_cache = {}


def _build_program(ntk, slots=SLOTS, debug=False, prec="fp16"):
    """Build one SPMD bass program for slot tile counts ntk (tuple of len slots)."""
    import concourse.bass as bass
    import concourse.bacc as bacc
    import concourse.mybir as mybir
    from concourse import tile, masks

    dt = mybir.dt.float32
    dtb = {
        "fp16": mybir.dt.float16,
        "bf16": mybir.dt.bfloat16,
        "fp32": mybir.dt.float32,
    }[prec]
    AF = mybir.ActivationFunctionType
    OP = mybir.AluOpType
    AX = mybir.AxisListType

    nc = bacc.Bacc(
        "TRN2",
        target_bir_lowering=False,
        debug=debug,
        num_devices=NCORES,
    )

    henc_in = nc.declare_dram_parameter("henc", [slots, L_FULL, H], dtb, isOutput=False)
    # pre-transposed h_enc, host layout [slot, l-tile, p=h%128, (j=h//128, l%128)]
    hencT_in = nc.declare_dram_parameter(
        "hencT", [slots, L_FULL // P, P, H], dtb, isOutput=False
    )
    mask_in = nc.declare_dram_parameter("maskT", [slots, P, L_FULL // P], dt, isOutput=False)
    beta_in = nc.declare_dram_parameter("beta", [slots, P, A], dt, isOutput=False)
    wc_in = nc.declare_dram_parameter("wc", [P, (H // P) * A], dtb, isOutput=False)
    wkT_in = nc.declare_dram_parameter("wkT", [P, (H // P) * E], dt, isOutput=False)
    vb_in = nc.declare_dram_parameter("vb", [P, A], dt, isOutput=False)
    ones_in = nc.declare_dram_parameter("onesr", [1, P], dt, isOutput=False)
    wout = nc.declare_dram_parameter("wout", [slots, L_FULL // P, P], dt, isOutput=True)
    ctx_out = nc.declare_dram_parameter("ctxout", [slots, E], dt, isOutput=True)

    with tile.TileContext(nc) as tc:
        with (
            tc.tile_pool(name="const", bufs=1) as constp,
            tc.tile_pool(name="x", bufs=11) as xpool,
            tc.tile_pool(name="xT", bufs=4) as xTpool,
            tc.tile_pool(name="th", bufs=3) as thpool,
            tc.tile_pool(name="junk", bufs=2) as junkpool,
            tc.tile_pool(name="scol", bufs=2) as scolp,
            tc.tile_pool(name="small", bufs=8) as smallp,
            tc.tile_pool(name="psS", bufs=3, space="PSUM") as psS,
            tc.tile_pool(name="psU", bufs=1, space="PSUM") as psU,
            tc.tile_pool(name="psE", bufs=2, space="PSUM") as psE,
        ):
            # ---- persistent constants ----
            wc_sb = constp.tile([P, (H // P) * A], dtb)
            nc.sync.dma_start(wc_sb[:], wc_in[:])
            wkT_sb = constp.tile([P, (H // P) * E], dt)
            nc.sync.dma_start(wkT_sb[:], wkT_in[:])
            v_sb = constp.tile([P, A], dt)
            nc.sync.dma_start(v_sb[:], vb_in[:])
            ones_sb = constp.tile([1, P], dt)
            nc.sync.dma_start(ones_sb[:], ones_in[:])
            ident = constp.tile([P, P], dt)
            masks.make_identity(nc, ident[:])
            ones_col = constp.tile([P, 1], dt)
            nc.vector.memset(ones_col[:], 1.0)
            u_all = constp.tile([slots, H], dt)

            for k in range(slots):
                nt = ntk[k]
                beta_sb = smallp.tile([P, A], dt, tag="beta")
                nc.sync.dma_start(beta_sb[:], beta_in[k])
                maskT_sb = smallp.tile([P, L_FULL // P], dt, tag="mask")
                nc.sync.dma_start(maskT_sb[:], mask_in[k])

                scol = scolp.tile([P, nt], dt, tag="scol")
                xts = []   # (tile, col_offset) per l-tile, for the u-pass
                xTs = []
                for t2 in range((nt + 1) // 2):
                    w2 = min(2, nt - t2 * 2)
                    x2 = xpool.tile([P, w2 * H], dtb, tag="x")
                    nc.sync.dma_start(
                        x2.rearrange("p (two h) -> p two h", two=w2),
                        henc_in[k, t2 * 2 * P : (t2 * 2 + w2) * P, :].rearrange(
                            "(two p) h -> p two h", p=P
                        ),
                    )
                    xT2 = xTpool.tile([P, w2 * H], dtb, tag="xT")
                    nc.sync.dma_start(
                        xT2.rearrange("p (two h) -> p two h", two=w2),
                        hencT_in[k, t2 * 2 : t2 * 2 + w2].rearrange(
                            "two p h -> p two h"
                        ),
                    )
                    for tl in range(w2):
                        xts.append((x2, tl * H))
                        xTs.append((xT2, tl * H))
                for t in range(nt):
                    xT, xoff = xTs[t]
                    # score matmul: S = beta (rank-1) + sum_j xT_j.T @ Wc_j
                    s_ps = psS.tile([P, A], dt, tag="s")
                    for j in range(H // P):
                        nc.tensor.matmul(
                            s_ps[:],
                            xT[:, xoff + j * P : xoff + (j + 1) * P],
                            wc_sb[:, j * A : (j + 1) * A],
                            start=(j == 0),
                            stop=(j == H // P - 1),
                        )
                    nc.vector.scalar_tensor_tensor(
                        out=s_ps[:], in0=s_ps[:], scalar=0.0, in1=beta_sb[:],
                        op0=OP.add, op1=OP.add,
                    )
                    th = thpool.tile([P, A], dt, tag="th")
                    nc.scalar.activation(th[:], s_ps[:], AF.Tanh)
                    junk = junkpool.tile([P, A], dt, tag="junk")
                    nc.vector.scalar_tensor_tensor(
                        out=junk[:],
                        in0=th[:],
                        scalar=0.0,
                        in1=v_sb[:],
                        op0=OP.add,
                        op1=OP.mult,
                        accum_out=scol[:, t : t + 1],
                    )

                # ---- slot epilogue: masked softmax over [P, nt] ----
                nc.vector.scalar_tensor_tensor(
                    out=scol[:], in0=scol[:], scalar=0.0, in1=maskT_sb[:, :nt],
                    op0=OP.add, op1=OP.add,
                )
                colmax = smallp.tile([P, 1], dt, tag="colmax")
                nc.vector.tensor_reduce(colmax[:], scol[:], axis=AX.X, op=OP.max)
                tmax_ps = psE.tile([1, P], dt, tag="e")
                nc.tensor.transpose(tmax_ps[:], colmax[:], ident[:])
                negm = smallp.tile([1, 1], dt, tag="negm")
                nc.vector.tensor_reduce(negm[:], tmax_ps[:], axis=AX.X, op=OP.max, negate=True)
                negm_ps = psE.tile([P, 1], dt, tag="e")
                nc.tensor.matmul(negm_ps[:], ones_sb[:], negm[:], start=True, stop=True)
                negm128 = smallp.tile([P, 1], dt, tag="negm128")
                nc.scalar.copy(negm128[:], negm_ps[:])

                pmat = scolp.tile([P, nt], dt, tag="pmat")
                rowsum = smallp.tile([P, 1], dt, tag="rowsum")
                nc.scalar.activation(
                    pmat[:], scol[:], AF.Exp, bias=negm128[:], scale=1.0,
                    accum_out=rowsum[:],
                )
                z_ps = psE.tile([1, 1], dt, tag="e")
                nc.tensor.matmul(z_ps[:], rowsum[:], ones_col[:], start=True, stop=True)
                z_sb = smallp.tile([1, 1], dt, tag="z_sb")
                nc.scalar.copy(z_sb[:], z_ps[:])
                rz = smallp.tile([1, 1], dt, tag="rz")
                nc.vector.reciprocal(rz[:], z_sb[:])
                rz_ps = psE.tile([P, 1], dt, tag="e")
                nc.tensor.matmul(rz_ps[:], ones_sb[:], rz[:], start=True, stop=True)
                rz128 = smallp.tile([P, 1], dt, tag="rz128")
                nc.scalar.copy(rz128[:], rz_ps[:])

                wn = scolp.tile([P, nt], dt, tag="wn")
                nc.vector.tensor_scalar_mul(wn[:], pmat[:], rz128[:])
                wT_ps = psE.tile([nt, P], dt, tag="e")
                nc.tensor.matmul(wT_ps[:], wn[:], ident[:], start=True, stop=True, is_transpose=True)
                wT_sb = smallp.tile([nt, P], dt, tag="wT_sb")
                nc.scalar.copy(wT_sb[:], wT_ps[:])
                nc.sync.dma_start(wout[k, :nt, :], wT_sb[:])

                # ---- u accumulation: u = sum_t P[:,t].T @ x_t  (unnormalized) ----
                pmat16 = scolp.tile([P, nt], dtb, tag="pmat16")
                nc.vector.tensor_copy(pmat16[:], pmat[:])
                u_ps = [
                    psU.tile([1, E], dt, tag=f"u{half}", name=f"u_ps{half}_{k}")
                    for half in range(2)
                ]
                for t in range(nt):
                    xn, noff = xts[t]
                    for half in range(2):
                        nc.tensor.matmul(
                            u_ps[half][:],
                            pmat16[:, t : t + 1],
                            xn[:, noff + half * E : noff + (half + 1) * E],
                            start=(t == 0),
                            stop=(t == nt - 1),
                        )
                u_row = smallp.tile([1, H], dt, tag="u_row")
                for half in range(2):
                    nc.scalar.activation(
                        u_row[:, half * E : (half + 1) * E],
                        u_ps[half][:],
                        AF.Copy,
                        scale=rz[:],
                    )
                nc.sync.dma_start(u_all[k : k + 1, :], u_row[:])

            # ---- ctx epilogue: ctx = (U @ WkT) scaled rows already ----
            ctx_ps = psE.tile([slots, E], dt, tag="e")
            for j in range(H // P):
                ut_ps = psE.tile([P, slots], dt, tag="e")
                nc.tensor.matmul(
                    ut_ps[:], u_all[:, j * P : (j + 1) * P], ident[:slots, :slots],
                    start=True, stop=True, is_transpose=True,
                )
                utj = smallp.tile([P, slots], dt, tag="utj")
                nc.vector.tensor_copy(utj[:], ut_ps[:])
                nc.tensor.matmul(
                    ctx_ps[:], utj[:], wkT_sb[:, j * E : (j + 1) * E],
                    start=(j == 0), stop=(j == H // P - 1),
                )
            ctx_sb = constp.tile([slots, E], dt)
            nc.scalar.copy(ctx_sb[:], ctx_ps[:])
            nc.sync.dma_start(ctx_out[:], ctx_sb[:])

    nc.compile()
    return nc


def kernel(h_dec, h_enc, src_lens, Wq, bq, Wk, bk, Wh, bh, v):
    import os
    import ml_dtypes
    from concourse.bass_utils import run_bass_kernel_spmd

    h_dec = np.asarray(h_dec, dtype=np.float32)
    h_enc = np.ascontiguousarray(np.asarray(h_enc, dtype=np.float32))
    src_lens_np = np.asarray(src_lens)
    Wq = np.asarray(Wq, dtype=np.float32); bq = np.asarray(bq, dtype=np.float32)
    Wk = np.asarray(Wk, dtype=np.float32); bk = np.asarray(bk, dtype=np.float32)
    Wh = np.asarray(Wh, dtype=np.float32); bh = np.asarray(bh, dtype=np.float32)
    v = np.asarray(v, dtype=np.float32)

    Wh_q, Wh_k = Wh[:, :E], Wh[:, E:]
    Wc = np.ascontiguousarray(
        (Wk.T.astype(np.float64) @ Wh_k.T.astype(np.float64)).astype(np.float32)
    )  # [H, A]
    qe = h_dec.astype(np.float64) @ Wq.T.astype(np.float64) + bq.astype(np.float64)
    beta = (
        qe @ Wh_q.T.astype(np.float64)
        + Wh_k.astype(np.float64) @ bk.astype(np.float64)
        + bh.astype(np.float64)
    ).astype(np.float32)  # [N, A]
    WkT = np.ascontiguousarray(Wk.T)  # [H, E]

    lens = np.clip(src_lens_np.astype(np.int64), 1, L_FULL)
    nt = np.ceil(lens / P).astype(np.int64)  # [N] tiles per example

    # deal examples (sorted by nt desc) into slots: slot k <- ranks [8k, 8k+8)
    order = np.argsort(-nt, kind="stable")
    assign = order.reshape(SLOTS, NCORES)          # assign[k, c] = example id
    ntk = tuple(int(nt[assign[k]].max()) for k in range(SLOTS))

    prec = os.environ.get("KERNEL_PREC", "fp16")
    key = (ntk, prec)
    if key not in _prog_cache:
        _prog_cache[key] = _build_program(ntk, prec=prec)
    nc = _prog_cache[key]

    # mask[n]: [P, L/P] with mask[p, t] = 0 if (t*P + p) < len else NEG
    l_idx = np.arange(L_FULL).reshape(L_FULL // P, P).T  # [P, L/P]
    mmdt = {"fp16": np.float16, "bf16": ml_dtypes.bfloat16, "fp32": np.float32}[prec]
    wc_dev = np.ascontiguousarray(
        Wc.reshape(H // P, P, A).transpose(1, 0, 2).reshape(P, (H // P) * A)
    ).astype(mmdt)
    wkT_dev = np.ascontiguousarray(
        WkT.reshape(H // P, P, E).transpose(1, 0, 2).reshape(P, (H // P) * E)
    )
    v_dev = np.ascontiguousarray(np.broadcast_to(v, (P, A)))
    ones_dev = np.ones((1, P), dtype=np.float32)

    in_maps = []
    for c in range(NCORES):
        ex = assign[:, c]  # [SLOTS]
        maskT = np.where(l_idx[None, :, :] < lens[ex][:, None, None], 0.0, NEG).astype(np.float32)
        in_maps.append({
            "henc": np.ascontiguousarray(h_enc[ex]).astype(mmdt),
            "hencT": np.ascontiguousarray(
                h_enc[ex]
                .reshape(SLOTS, L_FULL // P, P, H // P, P)
                .transpose(0, 1, 4, 3, 2)
                .reshape(SLOTS, L_FULL // P, P, H)
            ).astype(mmdt),
            "maskT": np.ascontiguousarray(maskT),
            "beta": np.ascontiguousarray(
                np.broadcast_to(beta[ex][:, None, :], (SLOTS, P, A))
            ),
            "wc": wc_dev,
            "wkT": wkT_dev,
            "vb": v_dev,
            "onesr": ones_dev,
        })

    trace_dir = os.environ.get("KERNEL_TRACE_DIR")
    if trace_dir:
        import axon_profile_shim
        axon_profile_shim.install()
        kr = run_bass_kernel_spmd(
            nc, in_maps, list(range(NCORES)), trace=True, tmpdir=trace_dir
        )
        global LAST_RESULTS
        LAST_RESULTS = kr
        print(f"HW exec time: {kr.exec_time_ns} ns (mean {kr.mean_exec_time_ns})")
        res = kr.results
    else:
        res = run_bass_kernel_spmd(nc, in_maps, list(range(NCORES))).results

    ctx = np.zeros((N_FULL, E), dtype=np.float32)
    weights = np.zeros((N_FULL, L_FULL), dtype=np.float32)
    for c in range(NCORES):
        w_c = res[c]["wout"].reshape(SLOTS, L_FULL)
        ctx_c = res[c]["ctxout"]
        for k in range(SLOTS):
            n = assign[k, c]
            m = ntk[k] * P
            weights[n, :m] = w_c[k, :m]
            ctx[n] = ctx_c[k] + bk
    return ctx, weights
